# revision 1
# baseline (speedup 1.0000x reference)
"""CLUB loss kernel for Trainium2, 8 NeuronCores, data-parallel over batch.

Math (see reference): two MLPs over modal_a produce mu and logvar; the loss
needs only two scalars:
  lld   = -0.5/B * sum_{i,h} (mu-b)^2 * iv
  bound = lld + 0.5/B * ( sum_h E[mu^2]_h * T0_h - 2 sum_h E[mu]_h * T1_h + T2 )
where iv = exp(-logvar).  Everything reduces to per-feature batch sums
  S1 = sum_i mu, S2 = sum_i mu^2, T0 = sum_i iv, T1 = sum_i mu*iv,
  T2 = sum_i mu^2*iv, P = sum_i (mu-b)^2*iv
which each core computes for its batch shard in one pass (no second pass over
mu, no device collectives -- the host combines 8 tiny [768,8] stat tensors).

Device layout: [feature, batch] everywhere, so all four matmuls run with
host-pretransposed operands and zero device transposes, biases fuse into
ScalarE activations, and batch sums are free-dim accumulations fused into
ScalarE (accum_out) / VectorE (scalar_tensor_tensor accum_out) instructions.

Schedule: L1v, L1m, then L2v/L2m interleaved at j-tile granularity (V one
tile ahead of M) so the per-tile statistics work (ACT+DVE+POOL) spreads over
a 2x wider PE window and r[j] is ready before L2m[j]'s epilogue chain.  All
activation funcs used (Tanh/Exp/Identity/Square) live in the single
`exp_and_others` ACT table set -- no table reloads.
"""

import numpy as np
import ml_dtypes

import concourse.bacc as bacc
import concourse.tile as tile
import concourse.mybir as mybir
from concourse.bass_utils import run_bass_kernel_spmd

B, H = 8192, 768
NCORES = 8
BS = B // NCORES          # 1024 rows per core
P = 128
KT = H // P               # 6 contraction tiles
JT = H // P               # 6 output-feature tiles
NI = 2                    # 512-wide moving chunks per matmul
IC = BS // NI             # 512

F32 = mybir.dt.float32
BF16 = mybir.dt.bfloat16
AF = mybir.ActivationFunctionType
ALU = mybir.AluOpType

_BF16 = ml_dtypes.bfloat16

_CACHE = {}
POOL_PRODUCTS = False
S2_ON_ACT = True
PROBE_NO_STATS = False
PSUM_SPLIT = False
DEDUP_LDW = False
FINE_HEAD = True
IC_OUTER = True


def _build(repeat=1, trace_sim=False, loop_n=None):
    nc = bacc.Bacc(trn_type="TRN2")

    aT_d = nc.dram_tensor("aT", [H, BS], BF16, kind="ExternalInput")
    bT_d = nc.dram_tensor("bT", [H, BS], BF16, kind="ExternalInput")
    w_d = {
        name: nc.dram_tensor(name, [H, H], BF16, kind="ExternalInput")
        for name in ("w1vT", "w2vT", "w1mT", "w2mT")
    }
    bias_d = nc.dram_tensor("biases", [P, 4 * JT], F32, kind="ExternalInput")
    stats_d = nc.dram_tensor("stats", [H, 8], F32, kind="ExternalOutput")

    with tile.TileContext(nc, trace_sim=trace_sim) as tc:
        with (
            tc.tile_pool(name="weights", bufs=1) as wp,
            tc.tile_pool(name="acts", bufs=1) as ap,
            tc.tile_pool(name="rot", bufs=3) as rot,
            tc.tile_pool(name="stat", bufs=3) as stp,
            tc.tile_pool(name="psum", bufs=4, space="PSUM") as pp,
        ):
            # --- persistent SBUF tensors -------------------------------------
            w_sb = {}
            for name in ("w1vT", "w1mT", "w2vT", "w2mT"):
                w_sb[name] = wp.tile([P, KT * H], BF16, name=f"{name}_sb")
            aT_sb = ap.tile([P, KT * BS], BF16, name="aT_sb")
            bT_sb = ap.tile([P, KT * BS], BF16, name="bT_sb")
            h1v_sb = ap.tile([P, JT * BS], BF16, name="h1v_sb")
            lv_sb = ap.tile([P, JT * BS], BF16, name="lv_sb")
            h1m_sb = ap.tile([P, JT * BS], BF16, name="h1m_sb")
            r_sb = ap.tile([P, JT * BS], BF16, name="r_sb")
            bias_sb = ap.tile([P, 4 * JT], F32, name="bias_sb")

            # --- input DMAs (phase order; interleave w1v/aT so the first
            # matmul's operands land first; kt=0 split fine so the j=0
            # k-chain can start as early as possible) -------------------------
            if FINE_HEAD:
                nc.sync.dma_start(w_sb["w1vT"][:, 0:H], w_d["w1vT"][0:P, :])
                nc.sync.dma_start(aT_sb[:, 0:IC], aT_d[0:P, 0:IC])
                nc.sync.dma_start(bias_sb, bias_d[:, :])
                nc.sync.dma_start(aT_sb[:, IC:BS], aT_d[0:P, IC:BS])
                rng0 = 1
            else:
                nc.sync.dma_start(bias_sb, bias_d[:, :])
                rng0 = 0
            for kt in range(rng0, KT):
                nc.sync.dma_start(
                    w_sb["w1vT"][:, kt * H:(kt + 1) * H],
                    w_d["w1vT"][kt * P:(kt + 1) * P, :])
                nc.sync.dma_start(
                    aT_sb[:, kt * BS:(kt + 1) * BS],
                    aT_d[kt * P:(kt + 1) * P, :])
            for kt in range(KT):
                nc.sync.dma_start(
                    w_sb["w1mT"][:, kt * H:(kt + 1) * H],
                    w_d["w1mT"][kt * P:(kt + 1) * P, :])
            for kt in range(KT):
                nc.sync.dma_start(
                    w_sb["w2vT"][:, kt * H:(kt + 1) * H],
                    w_d["w2vT"][kt * P:(kt + 1) * P, :])
            for kt in range(KT):
                nc.sync.dma_start(
                    w_sb["w2mT"][:, kt * H:(kt + 1) * H],
                    w_d["w2mT"][kt * P:(kt + 1) * P, :])
            for kt in range(KT):
                nc.sync.dma_start(
                    bT_sb[:, kt * BS:(kt + 1) * BS],
                    bT_d[kt * P:(kt + 1) * P, :])

            def matmul_tile(w, rhs_sb, j):
                """768-deep matmul for feature tile j.

                PSUM_SPLIT: two independent 1-bank psum tiles (8 banks in
                flight with bufs=8) read by two activation instructions;
                otherwise one 2-bank [128, BS] tile read by one instruction.
                """
                if PSUM_SPLIT:
                    pss = [pp.tile([P, IC], F32, tag=f"ps{_ic}", name=f"ps{_ic}",
                                   bufs=4) for _ic in range(NI)]
                    pss.append(None)
                else:
                    ps = pp.tile([P, BS], F32, tag="ps", name="ps")
                    pss = [ps[:, _ic * IC:(_ic + 1) * IC] for _ic in range(NI)]
                    pss.append(ps)
                if IC_OUTER:
                    for ic in range(NI):
                        for kt in range(KT):
                            lhsT = w[:, kt * H + j * P: kt * H + (j + 1) * P]
                            nc.tensor.matmul(
                                pss[ic], lhsT,
                                rhs_sb[:, kt * BS + ic * IC: kt * BS + (ic + 1) * IC],
                                start=(kt == 0), stop=(kt == KT - 1))
                else:
                    for kt in range(KT):
                        lhsT = w[:, kt * H + j * P: kt * H + (j + 1) * P]
                        for ic in range(NI):
                            nc.tensor.matmul(
                                pss[ic], lhsT,
                                rhs_sb[:, kt * BS + ic * IC: kt * BS + (ic + 1) * IC],
                                start=(kt == 0), stop=(kt == KT - 1))
                return pss

            def act_tile(ps, out_sb, j, bias_col, func, accum=None):
                bias_ap = bias_sb[:, bias_col: bias_col + 1]
                if PSUM_SPLIT:
                    for ic in range(NI):
                        nc.scalar.activation(
                            out_sb[:, j * BS + ic * IC: j * BS + (ic + 1) * IC],
                            ps[ic], func, bias=bias_ap)
                else:
                    nc.scalar.activation(
                        out_sb[:, j * BS:(j + 1) * BS],
                        ps[NI], func, bias=bias_ap, accum_out=accum)

            def l2v_tile(jv):
                """lv = tanh(W2v h1v + b2v); r = exp(-lv/2)."""
                ps = matmul_tile(w_sb["w2vT"], h1v_sb, jv)
                act_tile(ps, lv_sb, jv, 1 * JT + jv, AF.Tanh)
                if PROBE_NO_STATS:
                    return
                nc.scalar.activation(
                    r_sb[:, jv * BS:(jv + 1) * BS],
                    lv_sb[:, jv * BS:(jv + 1) * BS], AF.Exp, scale=-0.5)

            def l2m_tile(j):
                """mu = W2m h1m + b2m and all six fused statistics."""
                ps2 = matmul_tile(w_sb["w2mT"], h1m_sb, j)
                sa = stp.tile([P, 4], F32, tag="st_a")
                sv = stp.tile([P, 4], F32, tag="st_v")
                mu = rot.tile([P, BS], BF16, tag="mu")
                if PSUM_SPLIT:
                    for ic in range(NI):
                        nc.scalar.activation(
                            mu[:, ic * IC:(ic + 1) * IC], ps2[ic], AF.Identity,
                            bias=bias_sb[:, 3 * JT + j: 3 * JT + j + 1],
                            accum_out=sa[:, ic:ic + 1])
                else:
                    nc.scalar.activation(
                        mu, ps2[NI], AF.Identity,
                        bias=bias_sb[:, 3 * JT + j: 3 * JT + j + 1],
                        accum_out=sa[:, 0:1])
                if PROBE_NO_STATS:
                    nc.gpsimd.dma_start(stats_d[j * P:(j + 1) * P, 0:4], sa)
                    return

                rj = r_sb[:, j * BS:(j + 1) * BS]
                bj = bT_sb[:, j * BS:(j + 1) * BS]
                # T0 = sum r^2 (DVE stt)
                scr0 = rot.tile([P, BS], BF16, tag="scr0")
                nc.vector.scalar_tensor_tensor(
                    scr0, rj, 1.0, rj, ALU.bypass, ALU.mult,
                    accum_out=sv[:, 0:1])
                # d = mu - b, q2 = d*r, q1 = mu*r
                d = rot.tile([P, BS], BF16, tag="d")
                (nc.gpsimd if POOL_PRODUCTS else nc.vector).tensor_sub(d, mu, bj)
                q2 = rot.tile([P, BS], BF16, tag="q2")
                (nc.gpsimd if POOL_PRODUCTS else nc.vector).tensor_mul(q2, d, rj)
                q1 = rot.tile([P, BS], BF16, tag="q1")
                (nc.gpsimd if POOL_PRODUCTS else nc.vector).tensor_mul(q1, mu, rj)
                # P = sum q2^2 (DVE stt)
                scr2 = rot.tile([P, BS], BF16, tag="scr2")
                nc.vector.scalar_tensor_tensor(
                    scr2, q2, 1.0, q2, ALU.bypass, ALU.mult,
                    accum_out=sv[:, 2:3])
                # T1 = sum q1*r (DVE stt)
                scr1 = rot.tile([P, BS], BF16, tag="scr1")
                nc.vector.scalar_tensor_tensor(
                    scr1, q1, 1.0, rj, ALU.bypass, ALU.mult,
                    accum_out=sv[:, 1:2])
                # S2 = sum mu^2
                scr3 = rot.tile([P, BS], BF16, tag="scr3")
                if S2_ON_ACT:
                    nc.scalar.activation(scr3, mu, AF.Square,
                                         accum_out=sa[:, 3:4])
                else:
                    nc.vector.scalar_tensor_tensor(
                        scr3, mu, 1.0, mu, ALU.bypass, ALU.mult,
                        accum_out=sv[:, 3:4])
                # T2 = sum q1^2 (ACT Square accum)
                scr4 = rot.tile([P, BS], BF16, tag="scr4")
                nc.scalar.activation(scr4, q1, AF.Square,
                                     accum_out=sa[:, 2:3])
                nc.gpsimd.dma_start(stats_d[j * P:(j + 1) * P, 0:4], sa)
                nc.gpsimd.dma_start(stats_d[j * P:(j + 1) * P, 4:8], sv)

            def body():
                # --- L1v then L1m (both Tanh epilogues) ----------------------
                for j in range(JT):
                    ps = matmul_tile(w_sb["w1vT"], aT_sb, j)
                    act_tile(ps, h1v_sb, j, 0 * JT + j, AF.Tanh)
                for j in range(JT):
                    ps = matmul_tile(w_sb["w1mT"], aT_sb, j)
                    act_tile(ps, h1m_sb, j, 2 * JT + j, AF.Tanh)

                # --- interleaved L2, V one tile ahead of M -------------------
                l2v_tile(0)
                for j in range(JT - 1):
                    l2v_tile(j + 1)
                    l2m_tile(j)
                l2m_tile(JT - 1)

            if loop_n is not None:
                with tc.For_i(0, loop_n, 1,
                              hint_engines=(mybir.EngineType.PE,
                                            mybir.EngineType.Activation,
                                            mybir.EngineType.DVE,
                                            mybir.EngineType.Pool)):
                    for _rep in range(repeat):
                        body()
            else:
                for _rep in range(repeat):
                    body()

    nc.finalize()
    if DEDUP_LDW:
        _dedup_ldweights(nc)
    return nc


def _dedup_ldweights(nc):
    """Drop InstLdweights whose weights AP is identical to the previous PE
    weight load with only matmuls in between -- the weights are still
    resident in the PE array, and the redundant load costs ~53 ns of serial
    PE time each (bass emits one load per matmul with no reuse detection).
    """
    removed = 0
    for f in nc.m.functions:
        for bb in f.blocks:
            insts = list(bb.instructions)
            keep = []
            last_sig = None
            ok_since = True
            for ins in insts:
                eng = str(getattr(ins, "engine", ""))
                nm = type(ins).__name__
                if eng == "EngineType.PE":
                    if nm == "InstLdweights":
                        sig = str(ins.ins[0])
                        si = ins.sync_info
                        nw = len(si.on_wait) if si else 0
                        if sig == last_sig and ok_since and nw == 0:
                            removed += 1
                            continue
                        last_sig = sig
                        ok_since = True
                    elif nm != "InstMatmult":
                        ok_since = False
                        last_sig = None
                keep.append(ins)
            if len(keep) != len(insts):
                while len(bb.instructions):
                    bb.instructions.pop()
                for ins in keep:
                    bb.instructions.append(ins)
    return removed


def prepare_in_maps(modal_a, modal_b, W1m, b1m, W2m, b2m, W1v, b1v, W2v, b2v):
    w1mT = np.ascontiguousarray(W1m.astype(_BF16).T)
    w2mT = np.ascontiguousarray(W2m.astype(_BF16).T)
    w1vT = np.ascontiguousarray(W1v.astype(_BF16).T)
    w2vT = np.ascontiguousarray(W2v.astype(_BF16).T)
    bias_pack = np.zeros((P, 4 * JT), np.float32)
    for l, bias in enumerate((b1v, b2v, b1m, b2m)):
        bias_pack[:, l * JT:(l + 1) * JT] = np.asarray(
            bias, np.float32).reshape(JT, P).T

    a_bf = modal_a.astype(_BF16)
    b_bf = modal_b.astype(_BF16)
    in_maps = []
    for c in range(NCORES):
        rows = slice(c * BS, (c + 1) * BS)
        in_maps.append({
            "aT": np.ascontiguousarray(a_bf[rows].T),
            "bT": np.ascontiguousarray(b_bf[rows].T),
            "w1mT": w1mT, "w2mT": w2mT, "w1vT": w1vT, "w2vT": w2vT,
            "biases": bias_pack,
        })
    return in_maps


def combine_stats(stats_list):
    S1 = np.zeros(H); S2 = np.zeros(H); T0 = np.zeros(H)
    T1 = np.zeros(H); T2 = np.zeros(H); Ps = np.zeros(H)
    for st in stats_list:
        st = st.astype(np.float64)
        S1 += st[:, 0] + (st[:, 1] if PSUM_SPLIT else 0.0)
        T2 += st[:, 2]
        T0 += st[:, 4]
        T1 += st[:, 5]
        Ps += st[:, 6]
        S2 += st[:, 3] if S2_ON_ACT else st[:, 7]

    mu_mean = S1 / B
    mu_sq_mean = S2 / B
    lld = -0.5 / B * Ps.sum()
    neg_total = -0.5 * (mu_sq_mean @ T0 - 2.0 * (mu_mean @ T1) + T2.sum())
    bound = lld - neg_total / B
    return (np.float32(lld), np.float32(bound))


def kernel(modal_a, modal_b, W1m, b1m, W2m, b2m, W1v, b1v, W2v, b2v):
    if "nc" not in _CACHE:
        _CACHE["nc"] = _build()
    nc = _CACHE["nc"]

    in_maps = prepare_in_maps(modal_a, modal_b, W1m, b1m, W2m, b2m,
                              W1v, b1v, W2v, b2v)
    # One retry: a previously-wedged device surfaces as a runtime error on
    # the first execution and is reset by the failed attempt.
    try:
        res = run_bass_kernel_spmd(nc, in_maps, core_ids=list(range(NCORES)))
    except Exception:
        res = run_bass_kernel_spmd(nc, in_maps, core_ids=list(range(NCORES)))
    return combine_stats([res.results[c]["stats"] for c in range(NCORES)])

